# revision 41
# baseline (speedup 1.0000x reference)
"""Multi-head attention (B=4, L=2048, D=1024, H=16) on 8 TRN2 NeuronCores.

Sharding: core c handles batch b=c//2, query half qh=c%2 (1024 query tokens,
all heads, full 2048-key context). K/V projections are duplicated across the
2 cores sharing a batch; no cross-core communication needed.

v2 changes vs baseline:
  - Q/K projections run as fp8(e4m3) DoubleRow matmuls (0.5 cyc/row): host
    pre-packs X and W in [64, 2, *] interleaved-pair layout; PE proj cost
    halves (Q 32K cyc, K 64K cyc). V / out-proj stay bf16 (their error hits
    the output linearly; Q/K noise is damped by the /8 softmax scale).
  - Weights WQ8/WK8/WO persistent in SBUF, loaded once at startup (no wlp
    pool churn).
  - Startup reordered: biases -> xq8 (token halves) -> wq8[m0] -> wk8[m0] ->
    xk8 c-blocks interleaved with mask blocks; K m0 remaps per c-block so
    scores(0, 0..3) start after 1/4 of the K projection. First exp ~7us.
  - Mask multiplies for j%4==3 offloaded to GpSimd (Pool was idle; DVE was
    62% busy and within ~10% of becoming the pacer).
  - ScalarE exp is the wall (256 x [128,1024] tiles ~ 266us engine time);
    the schedule paces everything else around keeping it busy.

Per-core dataflow (unchanged):
  scores S.T[k,q] per (head, key-block j) via fp8 DoubleRow; E = exp(S/8) on
  ScalarE; E *= mask.T on DVE (+Pool for j%4==3); AV per (h, q-tile t) with a
  ones-column producing the softmax denominator; normalize on DVE; C.T via
  DMA-transpose; out = CT.T @ Wo.T + bo via ones-row bias matmul.
"""

import sys
import functools
from collections import deque

sys.path.insert(0, "/opt/trn_rl_repo")

import numpy as np
import ml_dtypes

BF16NP = ml_dtypes.bfloat16
F8NP = ml_dtypes.float8_e4m3

B, L, D, H, DK = 4, 2048, 1024, 16, 64
NCORES = 8
LQ = L // 2          # query tokens per core
NI = D // 128        # input-dim chunks
NM = D // 128        # dk-dim m-tiles (2 heads each)
NJ = L // 128        # key tiles
SLOT = DK + 1        # V slot width (64 cols + ones)
VW = H * SLOT        # 1040


def _build():
    import concourse.mybir as mybir
    import concourse.tile as tile
    from concourse import bacc

    dt = mybir.dt
    F32, BF, F8 = dt.float32, dt.bfloat16, dt.float8e4
    AF = mybir.ActivationFunctionType
    DR = mybir.MatmulPerfMode.DoubleRow

    nc = bacc.Bacc("TRN2", target_bir_lowering=False, debug=False,
                   num_devices=NCORES, dynamic_dma_scratch_size=1024)

    xq8_d = nc.dram_tensor("xq8", [128, NI * LQ], F8, kind="ExternalInput")
    xk8_d = nc.dram_tensor("xk8", [128, NI * L], F8, kind="ExternalInput")
    wq8_d = nc.dram_tensor("wq8", [128, NM * 1024], F8, kind="ExternalInput")
    wk8_d = nc.dram_tensor("wk8", [128, NM * 1024], F8, kind="ExternalInput")
    xv_d = nc.dram_tensor("xv", [NI, 128, L], BF, kind="ExternalInput")
    wv_d = nc.dram_tensor("wv", [NI, 128, D], BF, kind="ExternalInput")
    wo_d = nc.dram_tensor("wo", [NI, 128, D], BF, kind="ExternalInput")
    mt_d = nc.dram_tensor("maskt", [NJ, 128, LQ], BF, kind="ExternalInput")
    bq_d = nc.dram_tensor("bqt", [128, NM], F32, kind="ExternalInput")
    bk_d = nc.dram_tensor("bkt", [128, NM], F32, kind="ExternalInput")
    bo_d = nc.dram_tensor("bor", [1, D], BF, kind="ExternalInput")
    out_d = nc.dram_tensor("out", [NM, 128, D], BF, kind="ExternalOutput")

    keep = []

    def single(shape, dtyp, name):
        t, free = tc.tile(shape, dtyp, name=name)
        keep.append(free)
        return t

    with tile.TileContext(nc) as tc:
        # ---- persistent tiles ----
        VP = [single([128, VW], BF, f"vp{j}") for j in range(NJ)]
        CT = [single([128, LQ], BF, f"ct{m}") for m in range(NM)]
        MTA = single([128, NJ * LQ], BF, "mta")
        XQ8 = single([128, NI * LQ], F8, "xq8s")
        XK8 = single([128, NI * L], F8, "xk8s")
        WQ8 = single([128, NM * 1024], F8, "wq8s")
        WK8 = single([128, NM * 1024], F8, "wk8s")
        warm_sb = single([128, 256], BF, "warm_sb")
        bq_sb = single([128, NM], F32, "bq_sb")
        bk_sb = single([128, NM], F32, "bk_sb")
        bo_sb = single([1, D], BF, "bo_sb")
        ones_row = single([1, 128], BF, "ones_row")

        nc.vector.memset(ones_row[:], 1.0)
        nc.vector.memset(warm_sb[:], 0.0)

        # 4-d views over the DR-packed activations: [p, i2, r, t]
        xq8v = [XQ8[64 * i1:64 * i1 + 64, :].rearrange(
            "p (f r t) -> p f r t", r=2, t=LQ) for i1 in range(2)]
        xk8v = [XK8[64 * i1:64 * i1 + 64, :].rearrange(
            "p (f r t) -> p f r t", r=2, t=L) for i1 in range(2)]

        def wv8(W8, m, i):
            i2, i1 = i >> 1, i & 1
            return W8[64 * i1:64 * i1 + 64,
                      m * 1024 + i2 * 256:m * 1024 + i2 * 256 + 256
                      ].rearrange("p (r c) -> p r c", c=128)

        def xq_mv(i, c):
            i2 = i >> 1
            return xq8v[i & 1][:, i2, :, c * 512:(c + 1) * 512]

        def xk_mv(i, c):
            i2 = i >> 1
            return xk8v[i & 1][:, i2, :, c * 512:(c + 1) * 512]

        with (
            tc.tile_pool(name="q8p", bufs=2) as q8p,
            tc.tile_pool(name="k8p", bufs=2) as k8p,
            tc.tile_pool(name="f8p", bufs=2) as f8p,
            tc.tile_pool(name="ep", bufs=25) as ep,
            tc.tile_pool(name="wvp", bufs=2) as wvp,
            tc.tile_pool(name="xvp", bufs=2) as xvp,
            tc.tile_pool(name="cnp", bufs=2) as cnp,
            tc.tile_pool(name="rcp", bufs=2) as rcp,
            tc.tile_pool(name="sp", bufs=2, space="PSUM") as sp,
            tc.tile_pool(name="avp", bufs=2, space="PSUM") as avp,
            tc.tile_pool(name="wkp", bufs=2, space="PSUM") as wkp,
        ):
            Q8 = {}
            K8 = {}
            QF = {}
            KF = {}

            # ---------- projection emitters (fp8 DoubleRow) ----------
            def q_units(m):
                state = {}

                def unit(c, hi):
                    if c == 0 and hi == 0:
                        state["qf"] = f8p.tile([128, L], F8, tag="f",
                                               name=f"qf{m}")
                        QF[m] = state["qf"]
                    qf = state["qf"]
                    if hi == 0:
                        state[c] = wkp.tile([128, 512], F32, tag="k",
                                            name=f"psq{m}{c}")
                    ps = state[c]
                    for i in range(hi * 4, hi * 4 + 4):
                        nc.tensor.matmul(
                            ps[:], wv8(WQ8, m, i), xq_mv(i, c),
                            start=(i == 0), stop=(i == NI - 1),
                            perf_mode=DR, tile_position=(64 * (i & 1), 0))
                    if hi == 1:
                        cs = slice(c * 512, (c + 1) * 512)
                        nc.vector.tensor_scalar_add(qf[:, cs], ps[:],
                                                    bq_sb[:, m:m + 1])
                        if c == 1:
                            q8 = q8p.tile([64, 2 * LQ], F8, tag="q8",
                                          name=f"q8_{m}")
                            Q8[m] = q8
                            for p2 in range(2):
                                for i2 in range(2):
                                    s0 = 64 * p2 + 32 * i2
                                    nc.sync.dma_start(
                                        q8[32 * p2:32 * p2 + 32,
                                           i2 * LQ:(i2 + 1) * LQ],
                                        qf[s0:s0 + 32, 0:LQ])
                return [(f"Q{m}c{c}h{hi}", 1024, functools.partial(unit, c, hi))
                        for c in range(2) for hi in range(2)]

            def k_units(m, per_c_remap=False):
                state = {}

                def k_remap_c(c):
                    kf = state["kf"]
                    if "k8" not in state:
                        k8 = k8p.tile([64, 2 * L], F8, tag="k8",
                                      name=f"k8_{m}")
                        state["k8"] = k8
                        K8[m] = k8
                    k8 = state["k8"]
                    cs = slice(c * 512, (c + 1) * 512)
                    for p2 in range(2):
                        for i2 in range(2):
                            s0 = 64 * p2 + 32 * i2
                            nc.sync.dma_start(
                                k8[32 * p2:32 * p2 + 32,
                                   i2 * L + c * 512:i2 * L + (c + 1) * 512],
                                kf[s0:s0 + 32, cs])

                def k_remap_all():
                    kf = state["kf"]
                    k8 = k8p.tile([64, 2 * L], F8, tag="k8", name=f"k8_{m}")
                    state["k8"] = k8
                    K8[m] = k8
                    for p2 in range(2):
                        for i2 in range(2):
                            s0 = 64 * p2 + 32 * i2
                            nc.sync.dma_start(
                                k8[32 * p2:32 * p2 + 32,
                                   i2 * L:(i2 + 1) * L],
                                kf[s0:s0 + 32, :])

                def unit(c, hi):
                    if c == 0 and hi == 0:
                        state["kf"] = f8p.tile([128, L], F8, tag="f",
                                               name=f"kf{m}")
                        KF[m] = state["kf"]
                    kf = state["kf"]
                    cs = slice(c * 512, (c + 1) * 512)
                    if hi == 0:
                        state[c] = wkp.tile([128, 512], F32, tag="k",
                                            name=f"psk{m}{c}")
                    ps = state[c]
                    for i in range(hi * 4, hi * 4 + 4):
                        nc.tensor.matmul(
                            ps[:], wv8(WK8, m, i), xk_mv(i, c),
                            start=(i == 0), stop=(i == NI - 1),
                            perf_mode=DR, tile_position=(64 * (i & 1), 0))
                    if hi == 1:
                        nc.vector.tensor_scalar_add(kf[:, cs], ps[:],
                                                    bk_sb[:, m:m + 1])
                        if per_c_remap:
                            k_remap_c(c)
                        elif c == 3:
                            k_remap_all()

                return [(f"K{m}c{c}h{hi}", 1024,
                         functools.partial(unit, c, hi))
                        for c in range(4) for hi in range(2)]

            vheads = set()

            def v_group_units(h0):
                # 4 heads h0..h0+3 as two 2-head subpasses sharing the xv
                # tiles: units ordered (c, sub, jj) so each xv chunk is
                # loaded once per group; heads h0..h0+1 finish one unit
                # before the group end.
                wvs = {}
                xs_state = {}

                def load_ws(sub):
                    wt = wvp.tile([128, NI * 128], BF, tag="wv",
                                  name=f"wv{h0}_{sub}")
                    c0 = 64 * (h0 + 2 * sub)
                    nc.sync.dma_start(
                        wt[:].rearrange("p (i c) -> p i c", c=128),
                        wv_d.ap()[:, :, c0:c0 + 128].rearrange(
                            "i p c -> p i c"))
                    wvs[sub] = wt

                def load_xs(c):
                    xt = xvp.tile([128, NI * 512], BF, tag="xv",
                                  name=f"xv{h0}_{c}")
                    nc.sync.dma_start(
                        xt[:].rearrange("p (i t) -> p i t", t=512),
                        xv_d.ap()[:, :, c * 512:(c + 1) * 512].rearrange(
                            "i p t -> p i t"))
                    xs_state[c] = xt

                def preload():
                    load_ws(0)
                    load_xs(0)
                    load_xs(1)

                def unit(c, sub, jj):
                    if sub not in wvs:
                        load_ws(sub)
                    if c not in xs_state:
                        load_xs(c)
                    if sub == 0 and jj == 0 and c < 3 and c + 1 not in xs_state:
                        load_xs(c + 1)    # prefetch next chunk (2-buf pool)
                    xt = xs_state[c]
                    wt = wvs[sub]
                    j = c * 4 + jj
                    hh = h0 + 2 * sub
                    ps = wkp.tile([128, 512], F32, tag="k",
                                  name=f"psv{hh}_{j}")
                    for i in range(NI):
                        nc.tensor.matmul(
                            ps[:, 0:128],
                            xt[:, i * 512 + jj * 128:i * 512 + (jj + 1) * 128],
                            wt[:, i * 128:(i + 1) * 128],
                            start=(i == 0), stop=(i == NI - 1))
                    dst = VP[j][:].rearrange("p (h w) -> p h w", w=SLOT)[
                        :, hh:hh + 2, 0:DK]
                    src = ps[:, 0:128].rearrange("p (h w) -> p h w", w=DK)
                    nc.vector.tensor_copy(dst, src)
                    if h0 == 0 and sub == 0:
                        nc.vector.memset(VP[j][:, DK::SLOT], 1.0)
                    if c == 3 and jj == 3:
                        vheads.update((hh, hh + 1))

                units = [(f"V{h0 + 2 * sub}j{c * 4 + jj}", 1024,
                          functools.partial(unit, c, sub, jj))
                         for c in range(4) for sub in range(2)
                         for jj in range(4)]
                return units, preload

            # ---------- filler queue, deadline-ordered ----------
            # deadlines: qk(m) by window 2m; V heads h0..h0+1 by window h0+1
            v0_units, v0_preload = v_group_units(0)
            fillers = deque()
            fillers.extend(v0_units)                              # w1/w3
            fillers.extend(q_units(1))                            # w2
            fillers.extend(k_units(1))
            fillers.extend(q_units(2))                            # w4
            fillers.extend(k_units(2))
            fillers.extend(v_group_units(4)[0])                   # w5/w7
            fillers.extend(q_units(3))                            # w6
            fillers.extend(k_units(3))
            fillers.extend(q_units(4))                            # w8
            fillers.extend(k_units(4))
            fillers.extend(v_group_units(8)[0])                   # w9/w11
            fillers.extend(q_units(5))                            # w10
            fillers.extend(k_units(5))
            fillers.extend(q_units(6))                            # w12
            fillers.extend(k_units(6))
            fillers.extend(v_group_units(12)[0])                  # w13/w15
            fillers.extend(q_units(7))                            # w14
            fillers.extend(k_units(7))

            filler_debt = [0]

            def drain_fillers(budget):
                filler_debt[0] += budget
                while fillers and filler_debt[0] >= fillers[0][1]:
                    _, cost, fn = fillers.popleft()
                    filler_debt[0] -= cost
                    fn()

            def force_prefix(pred):
                while not pred():
                    assert fillers, "deadline unsatisfiable"
                    _, cost, fn = fillers.popleft()
                    fn()

            # ---------- startup ----------
            xq8s = XQ8[:].rearrange("p (f r t) -> p f r t", r=2, t=LQ)
            xq8d = xq8_d.ap().rearrange("p (f r t) -> p f r t", r=2, t=LQ)
            xk8s = XK8[:].rearrange("p (f r t) -> p f r t", r=2, t=L)
            xk8d = xk8_d.ap().rearrange("p (f r t) -> p f r t", r=2, t=L)

            def load_w8(dst, src_d, m):
                nc.sync.dma_start(dst[:, m * 1024:(m + 1) * 1024],
                                  src_d.ap()[:, m * 1024:(m + 1) * 1024])

            def load_mask4(g4):
                nc.sync.dma_start(
                    MTA[:, g4 * 4 * LQ:(g4 + 1) * 4 * LQ].rearrange(
                        "p (j t) -> p j t", t=LQ),
                    mt_d.ap()[4 * g4:4 * g4 + 4].rearrange("j p t -> p j t"))

            # critical-path DMA order (DMA payloads serialize on the shared
            # engine pool; HWDGE issues serialize at ~650ns each)
            nc.sync.dma_start(XQ8[:], xq8_d.ap())
            load_w8(WQ8, wq8_d, 0)
            nc.sync.dma_start(xk8s[:, :, :, 0:512], xk8d[:, :, :, 0:512])
            load_w8(WK8, wk8_d, 0)
            nc.sync.dma_start(bq_sb[:], bq_d.ap())
            nc.sync.dma_start(bk_sb[:], bk_d.ap())
            nc.sync.dma_start(xk8s[:, :, :, 512:1024], xk8d[:, :, :, 512:1024])
            nc.sync.dma_start(xk8s[:, :, :, 1024:1536], xk8d[:, :, :, 1024:1536])
            nc.sync.dma_start(xk8s[:, :, :, 1536:2048], xk8d[:, :, :, 1536:2048])

            # PE warmup: keep PE busy through the 3us p-state ramp while the
            # startup DMAs stream.
            for d in range(16):
                dps = avp.tile([128, 256], F32, tag="av", name=f"warm{d}")
                nc.tensor.matmul(dps[:], warm_sb[:, 0:128], warm_sb[:],
                                 start=True, stop=True)

            # Q m0 both units; K m0 unit c0 (rest inlined into window 0)
            qu0 = q_units(0)
            ku0 = k_units(0)
            for _, _, fn in qu0[0:2]:
                fn()
            for _, _, fn in ku0[0:2]:
                fn()
            for _, _, fn in qu0[2:4]:
                fn()

            # masks + first V inputs, need-ordered behind the critical loads
            v0_preload()           # wv(heads 0-1) + xv c0 + xv c1
            load_mask4(0)
            load_mask4(1)
            nc.sync.dma_start(bo_sb[:], bo_d.ap())
            load_mask4(2)
            load_mask4(3)

            # bulk loads paced into the window loop so they never sit ahead
            # of latency-critical mid-kernel DMAs on the serial SP queue
            def load_qk_w(m):
                def fn():
                    load_w8(WQ8, wq8_d, m)
                    load_w8(WK8, wk8_d, m)
                return fn

            dma_fillers = deque(load_qk_w(m) for m in range(1, NM))

            # ---------- attention ----------
            etiles = {}
            cn_tiles = {}

            def scores_unit(h, j):
                m, p2 = h // 2, h % 2
                base = 32 * p2
                s = sp.tile([128, LQ], F32, tag="s", name=f"s{h}_{j}")
                if h == 0:
                    # startup path: plain fp8 matmul straight from the
                    # projection tiles (2x PE cost on 8 units) so the first
                    # exps never wait on the DR-layout remap DMAs
                    for half in range(2):
                        hs = slice(half * 512, (half + 1) * 512)
                        nc.tensor.matmul(
                            s[:, hs], KF[0][0:64, j * 128:(j + 1) * 128],
                            QF[0][0:64, hs], start=True, stop=True,
                            tile_position=(0, 0))
                else:
                    k8v = K8[m][base:base + 32, :].rearrange(
                        "p (two l) -> p two l", two=2)
                    q8v = Q8[m][base:base + 32, :].rearrange(
                        "p (two l) -> p two l", two=2)
                    for half in range(2):
                        hs = slice(half * 512, (half + 1) * 512)
                        nc.tensor.matmul(
                            s[:, hs], k8v[:, :, j * 128:(j + 1) * 128],
                            q8v[:, :, hs], start=True, stop=True,
                            perf_mode=DR, tile_position=(base, 0))
                e = ep.tile([128, LQ], BF, tag="e", name=f"e{h}_{j}")
                nc.scalar.activation(e[:], s[:], AF.Exp, scale=0.125)
                # mask multiply: DVE mostly; every 4th tile on GpSimd (idle
                # engine; 3.6x slower per tile but off the DVE critical path)
                if j % 4 == 1:
                    nc.gpsimd.tensor_mul(e[:], e[:],
                                         MTA[:, j * LQ:(j + 1) * LQ])
                else:
                    nc.vector.tensor_mul(e[:], e[:],
                                         MTA[:, j * LQ:(j + 1) * LQ])
                etiles[(h, j)] = e

            def av_group(h, t):
                av = avp.tile([128, 512], F32, tag="av", name=f"av{h}_{t}")
                slot = slice(h * SLOT, (h + 1) * SLOT)
                for jj in range(NJ):
                    nc.tensor.matmul(
                        av[:, 0:SLOT],
                        etiles[(h, jj)][:, t * 128:(t + 1) * 128],
                        VP[jj][:, slot],
                        start=(jj == 0), stop=(jj == NJ - 1))
                m, p2 = h // 2, h % 2
                if p2 == 0 and t == 0:
                    cn_tiles[m] = cnp.tile([128, LQ], BF, tag="cn",
                                           name=f"cn{m}")
                cn = cn_tiles[m]
                rc = rcp.tile([128, 1], F32, tag="rc", name=f"rc{h}_{t}")
                nc.vector.reciprocal(rc[:], av[:, DK:DK + 1])
                nc.vector.tensor_scalar_mul(
                    cn[:, t * 128 + 64 * p2:t * 128 + 64 * p2 + 64],
                    av[:, 0:DK], rc[:])
                if p2 == 1 and m == NM - 1:
                    # last m: per-t transposes so out-proj unit t starts early
                    nc.sync.dma_start(CT[m][:, t * 128:(t + 1) * 128],
                                        cn[:, t * 128:(t + 1) * 128],
                                        transpose=True)
                elif p2 == 1 and t == 7:
                    nc.sync.dma_start(
                        CT[m][:].rearrange("p (t q) -> p t q", q=128),
                        cn[:], transpose=True)

            BUDGET = 1500
            wos = []
            for h in range(H):
                m = h // 2
                if m > 0:
                    force_prefix(lambda: m in K8 and m in Q8)
                av0 = 7 if h == 1 else 1    # first AV slot of this window
                for j in range(NJ):
                    scores_unit(h, j)
                    if h == 0:
                        if j == 0:
                            for _, _, fn in ku0[2:4]:   # K m0 c1
                                fn()
                        elif j == 2:
                            for _, _, fn in ku0[4:6]:   # K m0 c2
                                fn()
                        elif j == 4:
                            for _, _, fn in ku0[6:8]:   # K m0 c3
                                fn()
                    elif av0 <= j < av0 + 8:
                        if j == av0:
                            force_prefix(lambda: (h - 1) in vheads)
                        av_group(h - 1, j - av0)
                    if h == 0:
                        if j >= 6:
                            drain_fillers(1800)
                    elif av0 <= j < av0 + 8:
                        drain_fillers(900)
                    elif j >= 1:
                        drain_fillers(1800)
                    if dma_fillers and j in (6, 10, 14):
                        dma_fillers.popleft()()
                    if h == 13 and j == 5:
                        # reuse the xvp pool (V inputs consumed) for wo tiles
                        force_prefix(lambda: not fillers)
                        for half in range(2):
                            wt = xvp.tile([128, NI * 512], BF, tag="xv",
                                          name=f"wo{half}")
                            nc.sync.dma_start(
                                wt[:].rearrange("p (i d) -> p i d", d=D),
                                wo_d.ap()[4 * half:4 * half + 4].rearrange(
                                    "i p d -> p i d"))
                            for i in range(4):
                                wos.append(wt[:, i * D:(i + 1) * D])

            force_prefix(lambda: not fillers)
            for t in range(8):
                av_group(15, t)

            # ---------- output projection ----------
            for t in range(NM):
                po = sp.tile([128, D], F32, tag="s", name=f"po{t}")
                for half in range(2):
                    hs = slice(half * 512, (half + 1) * 512)
                    for c in range(NI):
                        nc.tensor.matmul(
                            po[:, hs], CT[c][:, t * 128:(t + 1) * 128],
                            wos[c][:, hs], start=(c == 0), stop=False)
                    nc.tensor.matmul(po[:, hs], ones_row[:], bo_sb[:, hs],
                                     start=False, stop=True)
                f = f8p.tile([128, D], BF, tag="f", name=f"f{t}")
                nc.scalar.activation(f[:], po[:], AF.Copy)
                nc.sync.dma_start(out_d.ap()[t], f[:])

    nc.compile()
    nc._keep_tile_frees = keep
    return nc


@functools.lru_cache(maxsize=1)
def _built():
    return _build()


def _pack_x_dr(xT):
    """xT [1024 chan, T] f32 -> [128, 8*T] fp8 DR-packed.

    chan = ((i2*2 + i1)*2 + r)*64 + p; partition = 64*i1 + p;
    free = (i2*2 + r)*T + t. Chunk i = i2*2+i1 sits at partition base
    64*(i&1), free block i>>1."""
    T = xT.shape[1]
    a = xT.reshape(4, 2, 2, 64, T)        # [i2, i1, r, p, t]
    a = a.transpose(1, 3, 0, 2, 4)        # [i1, p, i2, r, t]
    return np.ascontiguousarray(a.reshape(128, NI * T)).astype(F8NP)


def _pack_w_dr(WT):
    """WT [1024 in, 1024 out] f32 -> [128, 8192] fp8; free = (m, i2, r, c)."""
    a = WT.reshape(4, 2, 2, 64, 8, 128)   # [i2, i1, r, p, m, c]
    a = a.transpose(1, 3, 4, 0, 2, 5)     # [i1, p, m, i2, r, c]
    return np.ascontiguousarray(a.reshape(128, NM * 1024)).astype(F8NP)


def _prep_core(c, xq8s, xk8s, xvs, mask01T, wq8, wk8, wvt, wot, bqt, bkt, bor):
    b, qh = c // 2, c % 2
    qs = slice(qh * LQ, (qh + 1) * LQ)
    maskt = np.ascontiguousarray(mask01T[:, qs]).reshape(NJ, 128, LQ)
    return {
        "xq8": xq8s[c], "xk8": xk8s[b], "xv": xvs[b],
        "wq8": wq8, "wk8": wk8, "wv": wvt, "wo": wot,
        "maskt": maskt, "bqt": bqt, "bkt": bkt, "bor": bor,
    }


def kernel(q, k, v, attn_mask, Wq, bq, Wk, bk, Wv, bv, Wo, bo):
    from concourse import bass_utils

    nc = _built()

    q = np.asarray(q, np.float32)
    k = np.asarray(k, np.float32)
    v = np.asarray(v, np.float32)
    wq8 = _pack_w_dr(np.ascontiguousarray(np.asarray(Wq, np.float32).T))
    wk8 = _pack_w_dr(np.ascontiguousarray(np.asarray(Wk, np.float32).T))
    wvt = np.ascontiguousarray(np.asarray(Wv, np.float32).T).astype(BF16NP).reshape(NI, 128, D)
    wot = np.ascontiguousarray(np.asarray(Wo, np.float32).T).astype(BF16NP).reshape(NI, 128, D)
    mask01T = np.ascontiguousarray((np.asarray(attn_mask)[0, 0] != 0).T.astype(BF16NP))
    bqt = np.ascontiguousarray(np.asarray(bq, np.float32).reshape(NM, 128).T)
    bkt = np.ascontiguousarray(np.asarray(bk, np.float32).reshape(NM, 128).T)
    bo_eff = np.asarray(bo, np.float32) + np.asarray(Wo, np.float32) @ np.asarray(bv, np.float32)
    bor = bo_eff.astype(BF16NP).reshape(1, D)

    xq8s = [_pack_x_dr(np.ascontiguousarray(q[c // 2, (c % 2) * LQ:(c % 2 + 1) * LQ, :].T))
            for c in range(NCORES)]
    xk8s = [_pack_x_dr(np.ascontiguousarray(k[b].T)) for b in range(B)]
    xvs = [np.ascontiguousarray(v[b].T).astype(BF16NP).reshape(NI, 128, L)
           for b in range(B)]

    in_maps = [
        _prep_core(c, xq8s, xk8s, xvs, mask01T, wq8, wk8, wvt, wot, bqt, bkt, bor)
        for c in range(NCORES)
    ]
    res = bass_utils.run_bass_kernel_spmd(nc, in_maps, core_ids=list(range(NCORES)))

    out = np.empty((B, L, D), np.float32)
    for c in range(NCORES):
        b, qh = c // 2, c % 2
        out[b, qh * LQ:(qh + 1) * LQ, :] = (
            res.results[c]["out"].astype(np.float32).reshape(LQ, D))
    return out
